# revision 100
# baseline (speedup 1.0000x reference)
"""Fused self-attention + LayerNorm kernel for Trainium2 (8 NeuronCores).

Problem: B=8, S=2048, D=512 dense transformer attention layer.
  q = x@Wq + bq; k = x@Wk + bk; v = x@Wv + bv
  logits = q @ k^T / sqrt(D); attn = softmax(logits)  (mask is all-ones)
  out = LayerNorm(attn @ v) * gamma + beta

Sharding: batch-data-parallel, one batch element per core, no collectives.

Per-core kernel — every matmul is a DoubleRow fp8 matmul (2 contraction
rows per partition per cycle, 4x bf16 throughput under the cost model)
with 3-product error compensation: for operands split as hi+lo fp8 pairs,
a@b ~ ah@bh + ah@bl + al@bh reconstructs better than a single bf16
product at 0.75x the bf16 PE cost. Adjacent 128-blocks in the existing
[P, block, ...] tile layouts ARE the DoubleRow (partition, pair) packing
— no repacking is ever needed.

  - q/k fold: m = Wq@Wk^T/sqrt(D), so logits = (x@m)@x^T costs one
    projection (u) instead of two; the k-side bias term gk = Wk@bq enters
    as the exp's per-partition bias, the q-side term is softmax-invariant
  - host sends x, m, Wv as compensated fp8 hi/lo pairs (m prescaled by
    1024 and Wv by 16 to clear e4m3's subnormal cutoff; the exp scale and
    the v eviction undo them); u and attn are split on-device with the
    casts/residuals spread across ACT, DVE, and GPSIMD so no engine
    falls behind the PE's pace
  - logits computed TRANSPOSED, [k,q] per 128-k-block: exp(logitsT) is
    directly the stationary operand of attn@v — no PE transposes at all
  - softmax row-sums via 1-row ones-matmuls against the bf16 exp tmp
    (hwdecode PE makes them ~free); softmax normalization is folded into
    the LN epilogue analytically: c1 = (var_raw + eps*rowsum^2)^-0.5
  - epilogue split: DVE stats with the consume, ACT rsqrt (Exp(-.5*Ln))
    + store deferred past the next produce so the ACT FIFO never blocks
    exp evictions; one PSUM sums bank, column-region double-buffered
  - DMAs ordered/split around the single HWDGE queue to match PE
    consumption; dummy PE matmuls ramp the clock during the initial wait
  - last pair accumulates in idle psA banks (avoids a psO WAR), runs its
    q-chunks back-to-back, and column-splits the final accumulation and
    stores across two trigger queues to shorten the tail
"""

import sys

import numpy as np

_BASS_REPO = "/opt/trn_rl_repo"
if _BASS_REPO not in sys.path:
    sys.path.insert(0, _BASS_REPO)

import ml_dtypes  # noqa: E402

B, S, D = 8, 2048, 512
P = 128
NC_D = D // P  # 4 contraction chunks
SEG = 512
NSEG = S // SEG  # 4 free-dim segments
NBLK = S // P  # 16 k blocks
QP = 256  # q columns per produce (pair of 128-row chunks)
NPAIR = S // QP  # 8
EPS = 1e-5
BF = ml_dtypes.bfloat16
# host prescales so every fp8 hi/lo split sits in e4m3's normal range:
# M by 1024 (its entries AND u both clear subnormals; exp undoes it),
# Wv by 16 (undone in the v eviction)
USCALE = 1024.0
WVSCALE = 16.0
WARMUP_MM = 34  # dummy PE matmuls issued during the initial DMA wait

_cached_nc = {}
last_results = None  # BassKernelResults of the most recent run (for test.py)


def _build_nc(g1b0):
    import concourse.mybir as mybir
    from concourse import bacc
    from concourse.tile import TileContext

    BF16 = mybir.dt.bfloat16
    F32 = mybir.dt.float32
    FP8 = mybir.dt.float8e4
    Alu = mybir.AluOpType
    Act = mybir.ActivationFunctionType
    DR = mybir.MatmulPerfMode.DoubleRow

    nc = bacc.Bacc("TRN2", target_bir_lowering=False, debug=False)

    # host-split compensated fp8 pairs: x (reconstructs f32 x to ~0.03%,
    # better than bf16), m = Wq @ Wk^T * 1024/sqrt(D) (q/k folded into one
    # projection u = x@m; logits = u @ x^T), and Wv * 16. All projections
    # and logits then run as DoubleRow fp8 matmuls (2 contraction rows per
    # partition per cycle) with 3-product error compensation.
    # gk = Wk @ bq / sqrt(D) carries the only softmax-relevant bias term
    # (per-k, added pre-exp); the q-bias term is constant per row and
    # softmax-invariant, and the epilogue's c1 form is invariant to the
    # resulting rowsum rescale.
    xh_d = nc.declare_dram_parameter("xh", [D, S], FP8, isOutput=False)
    xl_d = nc.declare_dram_parameter("xl", [D, S], FP8, isOutput=False)
    mh_d = nc.declare_dram_parameter("mh", [D, D], FP8, isOutput=False)
    ml_d = nc.declare_dram_parameter("ml", [D, D], FP8, isOutput=False)
    wvh_d = nc.declare_dram_parameter("wvh", [D, D], FP8, isOutput=False)
    wvl_d = nc.declare_dram_parameter("wvl", [D, D], FP8, isOutput=False)
    gk_d = nc.declare_dram_parameter("gk", [D], BF16, isOutput=False)
    bv_d = nc.declare_dram_parameter("bv", [D], F32, isOutput=False)
    if not g1b0:
        gamma_d = nc.declare_dram_parameter("gamma", [D], F32, isOutput=False)
        beta_d = nc.declare_dram_parameter("beta", [D], F32, isOutput=False)
    out_d = nc.declare_dram_parameter("out", [S, D], F32, isOutput=True)

    import concourse.bass as bass

    def bcast(param_ap, parts=P):
        # [N] dram vector -> [parts, N] partition-broadcast AP
        return bass.AP(
            tensor=param_ap.tensor,
            offset=param_ap.offset,
            ap=[[0, parts]] + list(param_ap.ap),
        )

    with TileContext(nc) as tc:
        with (
            tc.tile_pool(name="pers", bufs=1) as pers,
            tc.tile_pool(name="attnp", bufs=2) as attnp,
            tc.tile_pool(name="attnHp", bufs=2) as attnHp,
            tc.tile_pool(name="attnLp", bufs=2) as attnLp,
            tc.tile_pool(name="work", bufs=4) as work,
            tc.tile_pool(name="small", bufs=6) as small,
            tc.tile_pool(name="psA", bufs=5, space="PSUM") as psA,
            tc.tile_pool(name="psO", bufs=2, space="PSUM") as psO,
            tc.tile_pool(name="psS", bufs=1, space="PSUM") as psS,
        ):
            # ---- persistent tiles ----
            mh_sb = pers.tile([P, NC_D, D], FP8, tag="mh")
            ml_sb = pers.tile([P, NC_D, D], FP8, tag="ml")
            wvh_sb = pers.tile([P, NC_D, D], FP8, tag="wvh")
            wvl_sb = pers.tile([P, NC_D, D], FP8, tag="wvl")
            xh_sb = pers.tile([P, NC_D, S], FP8, tag="xh")
            xl_sb = pers.tile([P, NC_D, S], FP8, tag="xl")

            # ---- input DMAs, ordered around the single HWDGE queue ----
            # Issue serializes at ~625ns/DMA and transfers serialize on the
            # DMA engines, so: order matches PE consumption — mh, then x
            # seg slabs (hi before lo, matching the product order inside
            # each accumulation group), ml, gk, then wv pair and bv.
            def seg_slab(dst, src, g):
                nc.sync.dma_start(
                    out=dst[:, :, g * SEG : (g + 1) * SEG],
                    in_=src.ap()[:, g * SEG : (g + 1) * SEG].rearrange(
                        "(c p) n -> p c n", p=P
                    ),
                )

            nc.sync.dma_start(
                out=mh_sb, in_=mh_d.ap().rearrange("(c p) n -> p c n", p=P)
            )
            seg_slab(xh_sb, xh_d, 0)
            seg_slab(xl_sb, xl_d, 0)
            nc.sync.dma_start(
                out=ml_sb, in_=ml_d.ap().rearrange("(c p) n -> p c n", p=P)
            )
            seg_slab(xh_sb, xh_d, 1)
            seg_slab(xl_sb, xl_d, 1)
            gk_sb = pers.tile([P, NC_D], BF16, tag="gk")
            nc.sync.dma_start(out=gk_sb, in_=gk_d.ap().rearrange("(c p) -> p c", p=P))
            for g in range(2, NSEG):
                seg_slab(xh_sb, xh_d, g)
                seg_slab(xl_sb, xl_d, g)
            nc.sync.dma_start(
                out=wvh_sb, in_=wvh_d.ap().rearrange("(c p) n -> p c n", p=P)
            )
            nc.sync.dma_start(
                out=wvl_sb, in_=wvl_d.ap().rearrange("(c p) n -> p c n", p=P)
            )
            bv_bc = pers.tile([P, D], F32, tag="bv")
            nc.sync.dma_start(out=bv_bc, in_=bcast(bv_d.ap()))
            if not g1b0:
                gamma_bc = pers.tile([P, D], F32, tag="gamma")
                nc.sync.dma_start(out=gamma_bc, in_=bcast(gamma_d.ap()))
                beta_bc = pers.tile([P, D], F32, tag="beta")
                nc.sync.dma_start(out=beta_bc, in_=bcast(beta_d.ap()))

            # PE clock warmup: the tensor engine ramps to full speed only
            # after ~3us of continuous execution. Chew through dummy 128-row
            # matmuls on a zeroed tile while the first input DMAs land.
            # wz is memset on GPSIMD (idle, short preamble) so warmup can
            # start ~0.5us in instead of waiting out the DVE preamble.
            wz = pers.tile([P, P], BF16, tag="wz")
            nc.gpsimd.memset(wz, 0.0)
            eps_sb = pers.tile([P, 1], F32, tag="eps")
            nc.vector.memset(eps_sb, EPS)
            # module-init const, ready at t=0 with no engine dependency;
            # rowsums stay bf16 1-row matmuls against the bf16 exp tmp (a
            # 1-row DoubleRow matmul fails walrus codegen): hi+lo matches
            # the bf16 tmp to ~0.03%, so sums stay consistent
            ones_sb = nc.const_aps.tensor(1.0, (P, 1), BF16)
            # dummy activation right at kernel start: pulls the one-time
            # 1.28us act-table load off the first exp eviction's critical
            # path — it runs concurrently with the input DMAs
            warm = pers.tile([P, 1], F32, tag="warm")
            nc.scalar.activation(out=warm, in_=eps_sb, func=Act.Exp)

            if WARMUP_MM:
                wps = psA.tile([P, SEG], F32, tag="mm", name="warmps")
                for _ in range(WARMUP_MM):
                    nc.tensor.matmul(wps[:, 0:P], wz, wz, start=True, stop=True)

            # ---- phase 1: u projection + gamma matvec, seg-outer ----
            # uT[d',s] (u = x@m): stationary = m chunk [d, d'-block],
            # moving = xT [d, s-seg]; accumulate over 4 d-chunks. seg-outer
            # so only xT's first 512 columns gate the start of compute.
            # The fp8 hi/lo split of u (DoubleRow logitsT operand) is
            # emitted per segment so produce(0) finds its g=0 blocks ready.
            uT_sb = pers.tile([P, NC_D, S], BF16, tag="uT")
            uh_sb = pers.tile([P, NC_D, S], FP8, tag="uh")
            ul_sb = pers.tile([P, NC_D, S], FP8, tag="ul")

            def u_prep(g, ms=range(NC_D)):
                # fp8 hi/lo split of u seg g; deferred (in halves, to keep
                # ACT's per-window load flat) until shortly before
                # produce(2g) consumes it, so phase-1 engines stay clear
                sl = slice(g * SEG, (g + 1) * SEG)
                for m in ms:
                    nc.scalar.activation(
                        out=uh_sb[:, m, sl], in_=uT_sb[:, m, sl], func=Act.Identity
                    )
                    eng = nc.vector if m % 2 == 0 else nc.gpsimd
                    eng.tensor_sub(ul_sb[:, m, sl], uT_sb[:, m, sl], uh_sb[:, m, sl])

            for g in range(NSEG):
                sl = slice(g * SEG, (g + 1) * SEG)
                pss = [
                    psA.tile([P, SEG], F32, tag="mm", name=f"pj{m}")
                    for m in range(NC_D)
                ]
                n = 0
                for s_sb, x_sb in ((mh_sb, xh_sb), (mh_sb, xl_sb), (ml_sb, xh_sb)):
                    for cp in range(NC_D // 2):
                        cs = slice(2 * cp, 2 * cp + 2)
                        n += 1
                        for m in range(NC_D):
                            nc.tensor.matmul(
                                pss[m],
                                s_sb[:, cs, m * P : (m + 1) * P],
                                x_sb[:, cs, sl],
                                start=(n == 1),
                                stop=(n == 3 * (NC_D // 2)),
                                perf_mode=DR,
                            )
                for m in range(NC_D):
                    # evict + cast to bf16; alternate ACT/DVE so two
                    # engines drain PSUM
                    if m % 2 == 0:
                        nc.scalar.activation(
                            out=uT_sb[:, m, sl], in_=pss[m], func=Act.Identity
                        )
                    else:
                        nc.vector.tensor_copy(out=uT_sb[:, m, sl], in_=pss[m])
            u_prep(0)
            # gamma[k] = x[k,:] @ gk: 1-row matmuls, ~free on the hwdecode
            # PE; added per-partition as the exp eviction's bias
            gam_sb = pers.tile([P, NBLK], F32, tag="gam")
            gps = psA.tile([P, SEG], F32, tag="mm", name="gps")
            for kb in range(NBLK):
                n = 0
                for x_sb in (xh_sb, xl_sb):
                    for c in range(NC_D):
                        n += 1
                        nc.tensor.matmul(
                            gps[:, kb : kb + 1],
                            x_sb[:, c, kb * P : (kb + 1) * P],
                            gk_sb[:, c : c + 1],
                            start=(n == 1),
                            stop=(n == 2 * NC_D),
                        )
            nc.vector.tensor_copy(out=gam_sb, in_=gps[:, 0:NBLK])
            # v[s,d']: stationary = xT block [d, s-block], moving = Wv [d, d']
            # v is consumed by fp8 DoubleRow attn@v as hi+lo (error-
            # compensated fp8 split): v_hi = fp8(v), v_lo = fp8(v - v_hi).
            # Engines: DVE computes v (bias add), ACT casts hi, GPSIMD the
            # lo residual — all off the PE's critical path.
            v_sb = pers.tile([P, NBLK, D], BF16, tag="v")
            vh_sb = pers.tile([P, NBLK, D], FP8, tag="vh")
            vl_sb = pers.tile([P, NBLK, D], FP8, tag="vl")
            for j in range(NBLK):
                ps = psA.tile([P, SEG], F32, tag="mm", name="vps")
                n = 0
                for x_sb, w_sb in ((xh_sb, wvh_sb), (xh_sb, wvl_sb), (xl_sb, wvh_sb)):
                    for cp in range(NC_D // 2):
                        cs = slice(2 * cp, 2 * cp + 2)
                        n += 1
                        nc.tensor.matmul(
                            ps,
                            x_sb[:, cs, j * P : (j + 1) * P],
                            w_sb[:, cs, :],
                            start=(n == 1),
                            stop=(n == 3 * (NC_D // 2)),
                            perf_mode=DR,
                        )
                # evict: undo the wv prescale + bias along free dim + cast
                nc.vector.scalar_tensor_tensor(
                    out=v_sb[:, j, :],
                    in0=ps,
                    scalar=1.0 / WVSCALE,
                    in1=bv_bc,
                    op0=Alu.mult,
                    op1=Alu.add,
                )
                nc.scalar.activation(
                    out=vh_sb[:, j, :], in_=v_sb[:, j, :], func=Act.Identity
                )
                # GPSIMD alone would overrun the v-loop window; DVE takes
                # every 4th residual
                eng = nc.vector if j % 4 == 3 else nc.gpsimd
                eng.tensor_sub(vl_sb[:, j, :], v_sb[:, j, :], vh_sb[:, j, :])



            # ---- phase 2: attention + layernorm, per 256-column q pair ----
            # Software-pipelined: produce pair p+1 (logitsT+exp) before
            # consuming pair p (attn@v + LN epilogue), so the PE never waits
            # on the ACT exp latency.
            def produce(p):
                # logitsT[k, q] per 128-k-block: stationary = xT block,
                # moving = uT pair-chunk. exp(logitsT) lands in a bf16 tmp,
                # then splits into fp8 hi+lo for DoubleRow attn@v. Engine
                # split keeps each under the PE's 427ns/k-block pace:
                # ACT exp, DVE hi cast, GPSIMD lo residual.
                tmp = attnp.tile([P, NBLK, QP], BF16, tag="attnT")
                aH = attnHp.tile([P, NBLK, QP], FP8, tag="aH")
                aL = attnLp.tile([P, NBLK, QP], FP8, tag="aL")
                qsl = slice(p * QP, (p + 1) * QP)
                for kb in range(NBLK):
                    lg = psA.tile([P, SEG], F32, tag="mm", name=f"lg{kb % 5}")
                    ksl = slice(kb * P, (kb + 1) * P)
                    n = 0
                    for cp in range(NC_D // 2):
                        cs = slice(2 * cp, 2 * cp + 2)
                        for sx, su in (
                            (xh_sb, uh_sb),
                            (xh_sb, ul_sb),
                            (xl_sb, uh_sb),
                        ):
                            n += 1
                            nc.tensor.matmul(
                                lg[:, 0:QP],
                                sx[:, cs, ksl],
                                su[:, cs, qsl],
                                start=(n == 1),
                                stop=(n == 3 * (NC_D // 2)),
                                perf_mode=DR,
                            )
                    # no max subtraction (|logits| < ~2.5 for this problem);
                    # gamma carries the k-bias term (zero for zero bq).
                    # M (hence u and the psum logits) is host-scaled by 64
                    # to keep u's fp8 hi/lo split out of the subnormal
                    # range; the exp's scale undoes it.
                    nc.scalar.activation(
                        out=tmp[:, kb, :],
                        in_=lg[:, 0:QP],
                        func=Act.Exp,
                        bias=gam_sb[:, kb : kb + 1],
                        scale=1.0 / USCALE,
                    )
                # hi/lo splits after the matmul+exp loop: ACT's exps keep
                # the psA-bank recycling pace, the casts drain afterwards
                # (deadline is the consume, a full pair-period later)
                if p < 2:
                    # pipeline fill: consume(p) runs j=0 first and needs
                    # only the left column-halves — emit those for every
                    # k-block before any right half so the first consume
                    # starts earlier. (GPSIMD still drains phase-1 v
                    # residuals here, so DVE carries the hi casts.)
                    for half in (slice(0, P), slice(P, QP)):
                        for kb in range(NBLK):
                            if kb % 2 == 0:
                                nc.vector.tensor_copy(
                                    out=aH[:, kb, half], in_=tmp[:, kb, half]
                                )
                            else:
                                nc.scalar.activation(
                                    out=aH[:, kb, half],
                                    in_=tmp[:, kb, half],
                                    func=Act.Identity,
                                )
                            eng = nc.gpsimd if kb % 2 == 0 else nc.vector
                            eng.tensor_sub(
                                aL[:, kb, half], tmp[:, kb, half], aH[:, kb, half]
                            )
                else:
                    for kb in range(NBLK):
                        if kb % 2 == 0:
                            nc.gpsimd.tensor_copy(out=aH[:, kb, :], in_=tmp[:, kb, :])
                        else:
                            nc.scalar.activation(
                                out=aH[:, kb, :], in_=tmp[:, kb, :], func=Act.Identity
                            )
                        nc.vector.tensor_sub(aL[:, kb, :], tmp[:, kb, :], aH[:, kb, :])
                return tmp, aH, aL

            # ---- epilogue, split in two stages ----
            # softmax normalization folded into LN:
            #   raw = attn_unnorm @ v; normalized x = raw / rowsum
            #   out = (raw - mean_raw) * c1 * gamma + beta, where
            #   c1 = (var_raw + eps*rowsum^2)^-0.5
            # (equals rstd(x)/rowsum analytically; eps*rowsum^2 keeps the
            # torch eps semantics). Stage A (DVE stats) is emitted with the
            # consume; stage B (ACT rsqrt via Exp(-0.5*Ln), final pass,
            # store) is deferred until after the NEXT produce so the ACT
            # FIFO never blocks that pair's exp evictions behind a
            # DVE-dependent Ln.
            def epi_a(p, j, out_ps, sums):
                sc = small.tile([P, 1], F32, tag="sc")
                nc.vector.tensor_copy(out=sc, in_=sums[:, j : j + 1])
                bst = small.tile([P, 6], F32, tag="bst")
                nc.vector.bn_stats(out=bst, in_=out_ps)
                mv = small.tile([P, 2], F32, tag="mv")
                nc.vector.bn_aggr(out=mv, in_=bst)
                t = small.tile([P, 1], F32, tag="t")
                nc.vector.tensor_scalar(
                    out=t,
                    in0=sc,
                    scalar1=sc,
                    scalar2=float(EPS),
                    op0=Alu.mult,
                    op1=Alu.mult,
                )
                return mv, t

            def epi_b(p, j, out_ps, mv, t, split, alt_queue=False):
                # rstd = (var + eps*s^2)^-0.5 as Exp(-0.5*Ln(.)) — the ACT
                # engine stays on the single ln+exp function table (a Sqrt
                # would force a 1.3us table reload twice per chunk)
                lnv = small.tile([P, 1], F32, tag="lnv")
                nc.scalar.activation(
                    out=lnv, in_=mv[:, 1:2], func=Act.Ln, bias=t, scale=1.0
                )
                c1 = small.tile([P, 1], F32, tag="c1")
                nc.scalar.activation(out=c1, in_=lnv, func=Act.Exp, scale=-0.5)

                row = (p * 2 + j) * P
                hw_ = D // split
                for h in range(split):
                    cols = slice(h * hw_, (h + 1) * hw_)
                    y = work.tile([P, hw_], F32, tag=f"y{h}", name=f"y{h}")
                    nc.vector.tensor_scalar(
                        out=y,
                        in0=out_ps[:, cols],
                        scalar1=mv[:, 0:1],
                        scalar2=c1,
                        op0=Alu.subtract,
                        op1=Alu.mult,
                    )
                    if not g1b0:
                        o1 = work.tile([P, hw_], F32, tag=f"o1{h}", name=f"o1{h}")
                        nc.vector.tensor_mul(o1, y, gamma_bc[:, cols])
                        y = work.tile([P, hw_], F32, tag=f"o{h}", name=f"o{h}")
                        nc.vector.tensor_add(y, o1, beta_bc[:, cols])
                    # alternate trigger queues on the tail so the final
                    # stores issue in parallel instead of serializing on SP
                    eng = nc.scalar if (alt_queue and h % 2 == 1) else nc.sync
                    eng.dma_start(out=out_d.ap()[row : row + P, cols], in_=y)

            # one persistent sums bank, column-region double-buffered by pair
            # parity so consecutive pairs' rowsum accumulations never share a
            # WAR dependency on the epilogue's read
            sums_all = psS.tile([P, 6], F32, tag="s")

            # attn@v in fp8 DoubleRow: each matmul contracts 256 k (two
            # adjacent kb blocks packed 2-per-partition, which is exactly
            # the layout of the [P, kb, ...] tiles) at 0.5 cycles/row.
            # Error-compensated: hi*vh + hi*vl + lo*vh ~ bf16 accuracy at
            # 0.75x the bf16 PE cost. Rowsums via 1-row DoubleRow matmuls
            # of (hi + lo) — exactly the weights the products apply.
            NK2 = NBLK // 2

            def consume_mm(p, tmp, aH, aL, outp, sums_col, j, cols=slice(0, D)):
                for kb2 in range(NK2):
                    ksl = slice(2 * kb2, 2 * kb2 + 2)
                    stH = aH[:, ksl, j * P : (j + 1) * P]
                    stL = aL[:, ksl, j * P : (j + 1) * P]
                    first = kb2 == 0
                    last = kb2 == NK2 - 1
                    nc.tensor.matmul(
                        outp, stH, vh_sb[:, ksl, cols],
                        start=first, stop=False, perf_mode=DR,
                    )
                    nc.tensor.matmul(
                        outp, stH, vl_sb[:, ksl, cols],
                        start=False, stop=False, perf_mode=DR,
                    )
                    nc.tensor.matmul(
                        outp, stL, vh_sb[:, ksl, cols],
                        start=False, stop=last, perf_mode=DR,
                    )
                    if sums_col is not None:
                        for i in (0, 1):
                            nc.tensor.matmul(
                                sums_col,
                                tmp[:, 2 * kb2 + i, j * P : (j + 1) * P],
                                ones_sb,
                                start=first and i == 0,
                                stop=last and i == 1,
                            )

            def consume_a(p, tmp, aH, aL):
                outps = [
                    psO.tile([P, D], F32, tag="out", name=f"out{j}") for j in (0, 1)
                ]
                sums = sums_all[:, (p % 2) * 2 : (p % 2) * 2 + 2]
                for j in (0, 1):
                    consume_mm(p, tmp, aH, aL, outps[j], sums[:, j : j + 1], j)
                state = []
                for j in (0, 1):
                    mv, t = epi_a(p, j, outps[j], sums)
                    state.append((outps[j], mv, t))
                return state

            # epi_b(p-2) is emitted BEFORE produce(p) so its psO-releasing
            # DVE reads run at the head of the produce window instead of
            # queueing behind the pair's hi/lo splits — consume(p-1) would
            # otherwise stall on the psO buffer WAR
            pend_attn = None  # produce(p) output awaiting consume
            pend_epi = None  # (p, state) awaiting epi_b
            for p in range(NPAIR):
                if pend_epi is not None:
                    ep, st = pend_epi
                    for j in (0, 1):
                        epi_b(ep, j, st[j][0], st[j][1], st[j][2], split=1)
                    pend_epi = None
                produced = produce(p)
                # u seg fp8 splits, half a segment per produce window so
                # ACT's cast load stays flat; emitted after produce so the
                # casts queue behind this pair's exps
                if p + 2 < NPAIR and p % 2 == 0:
                    u_prep((p + 2) // 2, range(0, 2))
                if p + 1 < NPAIR and (p + 1) % 2 == 0:
                    u_prep((p + 1) // 2, range(2, NC_D))
                if pend_attn is not None:
                    pend_epi = (p - 1, consume_a(p - 1, *pend_attn))
                pend_attn = produced
            ep, st = pend_epi
            for j in (0, 1):
                epi_b(ep, j, st[j][0], st[j][1], st[j][2], split=1)

            # last pair: accumulate into now-idle psA banks (no WAR against
            # the previous pair's psO epilogue reads), run the two q-chunks
            # back-to-back so chunk j=1's full epilogue+store overlaps chunk
            # j=0's matmuls, and column-halve j=0's accumulation so its
            # bn_stats mostly overlaps the final matmuls
            pl = NPAIR - 1
            tmpT, aH, aL = pend_attn
            sums = sums_all[:, (pl % 2) * 2 : (pl % 2) * 2 + 2]
            lout1 = psA.tile([P, D], F32, tag="mm", name="lout1")
            consume_mm(pl, tmpT, aH, aL, lout1, sums[:, 1:2], 1)
            mv, t = epi_a(pl, 1, lout1, sums)
            epi_b(pl, 1, lout1, mv, t, split=2)

            # j=0 accumulates its two column halves into SEPARATE tiles:
            # tile-level dependency tracking would otherwise see the h0
            # bn_stats (emitted between the halves so it overlaps the h1
            # matmuls) as conflicting with the h1 writes and stall the PE
            lsums = sums_all[:, 4:5]  # untouched column: no tracked deps
            # asymmetric halves: the small trailing piece minimizes the
            # post-PE bn_stats and the final store's transfer time
            HSPLIT = (slice(0, 384), slice(384, D))
            louts = [
                psA.tile([P, 384 if h == 0 else D - 384], F32, tag="mm", name=f"l0h{h}")
                for h in (0, 1)
            ]
            bst2 = small.tile([P, 12], F32, tag="bst2")
            for h in (0, 1):
                cols = HSPLIT[h]
                consume_mm(pl, tmpT, aH, aL, louts[h], lsums if h == 0 else None, 0, cols)
                if h == 0:
                    sc = small.tile([P, 1], F32, tag="sc")
                    nc.vector.tensor_copy(out=sc, in_=lsums)
                    t = small.tile([P, 1], F32, tag="t")
                    nc.vector.tensor_scalar(
                        out=t,
                        in0=sc,
                        scalar1=sc,
                        scalar2=float(EPS),
                        op0=Alu.mult,
                        op1=Alu.mult,
                    )
                nc.vector.bn_stats(out=bst2[:, h * 6 : (h + 1) * 6], in_=louts[h])
            mv = small.tile([P, 2], F32, tag="mv")
            nc.vector.bn_aggr(out=mv, in_=bst2)
            lnv = small.tile([P, 1], F32, tag="lnv")
            nc.scalar.activation(
                out=lnv, in_=mv[:, 1:2], func=Act.Ln, bias=t, scale=1.0
            )
            c1 = small.tile([P, 1], F32, tag="c1")
            nc.scalar.activation(out=c1, in_=lnv, func=Act.Exp, scale=-0.5)
            row = pl * 2 * P
            for h in (0, 1):
                cols = HSPLIT[h]
                hw_ = cols.stop - cols.start
                y = work.tile([P, hw_], F32, tag=f"y{h}", name=f"ly{h}")
                nc.vector.tensor_scalar(
                    out=y,
                    in0=louts[h],
                    scalar1=mv[:, 0:1],
                    scalar2=c1,
                    op0=Alu.subtract,
                    op1=Alu.mult,
                )
                if not g1b0:
                    o1 = work.tile([P, hw_], F32, tag=f"o1{h}", name=f"lo1{h}")
                    nc.vector.tensor_mul(o1, y, gamma_bc[:, cols])
                    y = work.tile([P, hw_], F32, tag=f"o{h}", name=f"lo{h}")
                    nc.vector.tensor_add(y, o1, beta_bc[:, cols])
                nc.sync.dma_start(out=out_d.ap()[row : row + P, cols], in_=y)

    # Force every ACT instruction onto the one table set that contains all
    # functions we use ({exp, ln, identity} ⊆ natural_log_exp_and_others).
    # The default chooser picks the FIRST set containing each function
    # (exp→set0, ln→set5), inserting a 1.28us table reload twice per
    # chunk. Entries must keep their positions (act_func_set_id is the
    # index), so unwanted sets are emptied rather than removed.
    import concourse.bacc as bacc_mod

    orig_get_tables = bacc_mod.get_activation_tables

    def pinned_tables(arch):
        out = {}
        for name, funcs in orig_get_tables(arch).items():
            out[name] = funcs if name == "natural_log_exp_and_others" else set()
        return out

    bacc_mod.get_activation_tables = pinned_tables
    try:
        nc.compile()
    finally:
        bacc_mod.get_activation_tables = orig_get_tables
    return nc


def _numpy_fallback(query, mask, Wq, bq, Wk, bk, Wv, bv, gamma, beta):
    q = query @ Wq + bq
    k = query @ Wk + bk
    v = query @ Wv + bv
    scale = 1.0 / np.sqrt(np.float32(q.shape[-1]))
    logits = np.einsum("bqd,bkd->bqk", q, k) * scale
    m = np.swapaxes(mask, 1, 2)
    logits = np.where(m, logits, np.float32(-1e9))
    logits = logits - logits.max(axis=2, keepdims=True)
    attn = np.exp(logits)
    attn = attn / attn.sum(axis=2, keepdims=True)
    out = np.einsum("bqk,bkd->bqd", attn, v)
    mu = out.mean(axis=-1, keepdims=True)
    var = out.var(axis=-1, keepdims=True)
    return (out - mu) / np.sqrt(var + 1e-5) * gamma + beta


def kernel(query, mask, Wq, bq, Wk, bk, Wv, bv, gamma, beta):
    global last_results
    from concourse.bass_utils import run_bass_kernel_spmd

    query = np.asarray(query, dtype=np.float32)
    mask = np.asarray(mask)
    Wq = np.asarray(Wq, dtype=np.float32)
    Wk = np.asarray(Wk, dtype=np.float32)
    Wv = np.asarray(Wv, dtype=np.float32)
    bq = np.asarray(bq, dtype=np.float32)
    bk = np.asarray(bk, dtype=np.float32)
    bv = np.asarray(bv, dtype=np.float32)
    gamma = np.asarray(gamma, dtype=np.float32)
    beta = np.asarray(beta, dtype=np.float32)

    if not mask.all():
        # General-mask path (never hit for this problem's all-ones mask).
        return _numpy_fallback(
            query, mask, Wq, bq, Wk, bk, Wv, bv, gamma, beta
        ).astype(np.float32)

    g1b0 = bool((gamma == 1.0).all() and (beta == 0.0).all())
    if g1b0 not in _cached_nc:
        _cached_nc[g1b0] = _build_nc(g1b0)
    nc = _cached_nc[g1b0]

    F8 = ml_dtypes.float8_e4m3fn

    def split8(a):
        hi = a.astype(F8)
        lo = (a - hi.astype(np.float64)).astype(F8)
        return hi, lo

    scale = 1.0 / np.sqrt(np.float64(D))
    # fold the two q/k projections into one: logits = x @ m @ x^T + gk-term
    m_f = (Wq.astype(np.float64) @ Wk.astype(np.float64).T) * (scale * USCALE)
    mh_b, ml_b = split8(m_f)
    wvh_b, wvl_b = split8(Wv.astype(np.float64) * WVSCALE)
    gk_b = ((Wk.astype(np.float64) @ bq.astype(np.float64)) * scale).astype(BF)

    in_maps = []
    for b in range(B):
        xT = np.ascontiguousarray(query[b].T).astype(np.float64)
        xh, xl = split8(xT)
        m = {
            "xh": xh,
            "xl": xl,
            "mh": mh_b,
            "ml": ml_b,
            "wvh": wvh_b,
            "wvl": wvl_b,
            "gk": gk_b,
            "bv": bv,
        }
        if not g1b0:
            m["gamma"] = gamma
            m["beta"] = beta
        in_maps.append(m)

    res = run_bass_kernel_spmd(nc, in_maps, core_ids=list(range(B)))
    last_results = res
    out = np.stack([res.results[b]["out"] for b in range(B)], axis=0)
    return out.astype(np.float32)


# revision 101
# speedup vs baseline: 1.0361x; 1.0361x over previous
"""Fused self-attention + LayerNorm kernel for Trainium2 (8 NeuronCores).

Problem: B=8, S=2048, D=512 dense transformer attention layer.
  q = x@Wq + bq; k = x@Wk + bk; v = x@Wv + bv
  logits = q @ k^T / sqrt(D); attn = softmax(logits)  (mask is all-ones)
  out = LayerNorm(attn @ v) * gamma + beta

Sharding: batch-data-parallel, one batch element per core, no collectives.

Per-core kernel — every matmul is a DoubleRow fp8 matmul (2 contraction
rows per partition per cycle, 4x bf16 throughput under the cost model)
with 3-product error compensation: for operands split as hi+lo fp8 pairs,
a@b ~ ah@bh + ah@bl + al@bh reconstructs better than a single bf16
product at 0.75x the bf16 PE cost. Adjacent 128-blocks in the existing
[P, block, ...] tile layouts ARE the DoubleRow (partition, pair) packing
— no repacking is ever needed.

  - q/k fold: m = Wq@Wk^T/sqrt(D), so logits = (x@m)@x^T costs one
    projection (u) instead of two; the k-side bias term gk = Wk@bq enters
    as the exp's per-partition bias, the q-side term is softmax-invariant
  - host sends x, m, Wv as compensated fp8 hi/lo pairs (m prescaled by
    1024 and Wv by 16 to clear e4m3's subnormal cutoff; the exp scale and
    the v eviction undo them); u and attn are split on-device with the
    casts/residuals spread across ACT, DVE, and GPSIMD so no engine
    falls behind the PE's pace
  - logits computed TRANSPOSED, [k,q] per 128-k-block: exp(logitsT) is
    directly the stationary operand of attn@v — no PE transposes at all
  - softmax row-sums via 1-row ones-matmuls against the bf16 exp tmp
    (hwdecode PE makes them ~free); softmax normalization is folded into
    the LN epilogue analytically: c1 = (var_raw + eps*rowsum^2)^-0.5
  - epilogue split: DVE stats with the consume, ACT rsqrt (Exp(-.5*Ln))
    + store deferred past the next produce so the ACT FIFO never blocks
    exp evictions; one PSUM sums bank, column-region double-buffered
  - DMAs ordered/split around the single HWDGE queue to match PE
    consumption; dummy PE matmuls ramp the clock during the initial wait
  - last pair accumulates in idle psA banks (avoids a psO WAR), runs its
    q-chunks back-to-back, and column-splits the final accumulation and
    stores across two trigger queues to shorten the tail
"""

import sys

import numpy as np

_BASS_REPO = "/opt/trn_rl_repo"
if _BASS_REPO not in sys.path:
    sys.path.insert(0, _BASS_REPO)

import ml_dtypes  # noqa: E402

B, S, D = 8, 2048, 512
P = 128
NC_D = D // P  # 4 contraction chunks
SEG = 512
NSEG = S // SEG  # 4 free-dim segments
NBLK = S // P  # 16 k blocks
QP = 256  # q columns per produce (pair of 128-row chunks)
NPAIR = S // QP  # 8
EPS = 1e-5
BF = ml_dtypes.bfloat16
# host prescales so every fp8 hi/lo split sits in e4m3's normal range:
# M by 1024 (its entries AND u both clear subnormals; exp undoes it),
# Wv by 16 (undone in the v eviction)
USCALE = 1024.0
WVSCALE = 16.0
WARMUP_MM = 34  # dummy PE matmuls issued during the initial DMA wait

_cached_nc = {}
last_results = None  # BassKernelResults of the most recent run (for test.py)


def _build_nc(g1b0):
    import concourse.mybir as mybir
    from concourse import bacc
    from concourse.tile import TileContext

    BF16 = mybir.dt.bfloat16
    F32 = mybir.dt.float32
    FP8 = mybir.dt.float8e4
    Alu = mybir.AluOpType
    Act = mybir.ActivationFunctionType
    DR = mybir.MatmulPerfMode.DoubleRow

    nc = bacc.Bacc("TRN2", target_bir_lowering=False, debug=False)

    # host-split compensated fp8 pairs: x (reconstructs f32 x to ~0.03%,
    # better than bf16), m = Wq @ Wk^T * 1024/sqrt(D) (q/k folded into one
    # projection u = x@m; logits = u @ x^T), and Wv * 16. All projections
    # and logits then run as DoubleRow fp8 matmuls (2 contraction rows per
    # partition per cycle) with 3-product error compensation.
    # gk = Wk @ bq / sqrt(D) carries the only softmax-relevant bias term
    # (per-k, added pre-exp); the q-bias term is constant per row and
    # softmax-invariant, and the epilogue's c1 form is invariant to the
    # resulting rowsum rescale.
    xh_d = nc.declare_dram_parameter("xh", [D, S], FP8, isOutput=False)
    xl_d = nc.declare_dram_parameter("xl", [D, S], FP8, isOutput=False)
    mh_d = nc.declare_dram_parameter("mh", [D, D], FP8, isOutput=False)
    ml_d = nc.declare_dram_parameter("ml", [D, D], FP8, isOutput=False)
    wvh_d = nc.declare_dram_parameter("wvh", [D, D], FP8, isOutput=False)
    wvl_d = nc.declare_dram_parameter("wvl", [D, D], FP8, isOutput=False)
    gk_d = nc.declare_dram_parameter("gk", [D], BF16, isOutput=False)
    bv_d = nc.declare_dram_parameter("bv", [D], F32, isOutput=False)
    if not g1b0:
        gamma_d = nc.declare_dram_parameter("gamma", [D], F32, isOutput=False)
        beta_d = nc.declare_dram_parameter("beta", [D], F32, isOutput=False)
    out_d = nc.declare_dram_parameter("out", [S, D], F32, isOutput=True)

    import concourse.bass as bass

    def bcast(param_ap, parts=P):
        # [N] dram vector -> [parts, N] partition-broadcast AP
        return bass.AP(
            tensor=param_ap.tensor,
            offset=param_ap.offset,
            ap=[[0, parts]] + list(param_ap.ap),
        )

    with TileContext(nc) as tc:
        with (
            tc.tile_pool(name="pers", bufs=1) as pers,
            tc.tile_pool(name="attnp", bufs=2) as attnp,
            tc.tile_pool(name="attnHp", bufs=2) as attnHp,
            tc.tile_pool(name="attnLp", bufs=2) as attnLp,
            tc.tile_pool(name="work", bufs=4) as work,
            tc.tile_pool(name="small", bufs=6) as small,
            tc.tile_pool(name="psA", bufs=5, space="PSUM") as psA,
            tc.tile_pool(name="psO", bufs=2, space="PSUM") as psO,
            tc.tile_pool(name="psS", bufs=1, space="PSUM") as psS,
        ):
            # ---- persistent tiles ----
            mh_sb = pers.tile([P, NC_D, D], FP8, tag="mh")
            ml_sb = pers.tile([P, NC_D, D], FP8, tag="ml")
            wvh_sb = pers.tile([P, NC_D, D], FP8, tag="wvh")
            wvl_sb = pers.tile([P, NC_D, D], FP8, tag="wvl")
            xh_sb = pers.tile([P, NC_D, S], FP8, tag="xh")
            xl_sb = pers.tile([P, NC_D, S], FP8, tag="xl")

            # ---- input DMAs, ordered around the single HWDGE queue ----
            # Issue serializes at ~625ns/DMA and transfers serialize on the
            # DMA engines, so: order matches PE consumption — mh, then x
            # seg slabs (hi before lo, matching the product order inside
            # each accumulation group), ml, gk, then wv pair and bv.
            def seg_slab(dst, src, g):
                nc.sync.dma_start(
                    out=dst[:, :, g * SEG : (g + 1) * SEG],
                    in_=src.ap()[:, g * SEG : (g + 1) * SEG].rearrange(
                        "(c p) n -> p c n", p=P
                    ),
                )

            nc.sync.dma_start(
                out=mh_sb, in_=mh_d.ap().rearrange("(c p) n -> p c n", p=P)
            )
            seg_slab(xh_sb, xh_d, 0)
            seg_slab(xl_sb, xl_d, 0)
            nc.sync.dma_start(
                out=ml_sb, in_=ml_d.ap().rearrange("(c p) n -> p c n", p=P)
            )
            seg_slab(xh_sb, xh_d, 1)
            seg_slab(xl_sb, xl_d, 1)
            gk_sb = pers.tile([P, NC_D], BF16, tag="gk")
            nc.sync.dma_start(out=gk_sb, in_=gk_d.ap().rearrange("(c p) -> p c", p=P))
            for g in range(2, NSEG):
                seg_slab(xh_sb, xh_d, g)
                seg_slab(xl_sb, xl_d, g)
            nc.sync.dma_start(
                out=wvh_sb, in_=wvh_d.ap().rearrange("(c p) n -> p c n", p=P)
            )
            nc.sync.dma_start(
                out=wvl_sb, in_=wvl_d.ap().rearrange("(c p) n -> p c n", p=P)
            )
            bv_bc = pers.tile([P, D], F32, tag="bv")
            nc.sync.dma_start(out=bv_bc, in_=bcast(bv_d.ap()))
            if not g1b0:
                gamma_bc = pers.tile([P, D], F32, tag="gamma")
                nc.sync.dma_start(out=gamma_bc, in_=bcast(gamma_d.ap()))
                beta_bc = pers.tile([P, D], F32, tag="beta")
                nc.sync.dma_start(out=beta_bc, in_=bcast(beta_d.ap()))

            # PE clock warmup: the tensor engine ramps to full speed only
            # after ~3us of continuous execution. Chew through dummy 128-row
            # matmuls on a zeroed tile while the first input DMAs land.
            # wz is memset on GPSIMD (idle, short preamble) so warmup can
            # start ~0.5us in instead of waiting out the DVE preamble.
            wz = pers.tile([P, P], BF16, tag="wz")
            nc.gpsimd.memset(wz, 0.0)
            eps_sb = pers.tile([P, 1], F32, tag="eps")
            nc.vector.memset(eps_sb, EPS)
            # module-init const, ready at t=0 with no engine dependency;
            # rowsums stay bf16 1-row matmuls against the bf16 exp tmp (a
            # 1-row DoubleRow matmul fails walrus codegen): hi+lo matches
            # the bf16 tmp to ~0.03%, so sums stay consistent
            ones_sb = nc.const_aps.tensor(1.0, (P, 1), BF16)
            # dummy activation right at kernel start: pulls the one-time
            # 1.28us act-table load off the first exp eviction's critical
            # path — it runs concurrently with the input DMAs
            warm = pers.tile([P, 1], F32, tag="warm")
            nc.scalar.activation(out=warm, in_=eps_sb, func=Act.Exp)

            if WARMUP_MM:
                wps = psA.tile([P, SEG], F32, tag="mm", name="warmps")
                for _ in range(WARMUP_MM):
                    nc.tensor.matmul(wps[:, 0:P], wz, wz, start=True, stop=True)

            # ---- phase 1: u projection + gamma matvec, seg-outer ----
            # uT[d',s] (u = x@m): stationary = m chunk [d, d'-block],
            # moving = xT [d, s-seg]; accumulate over 4 d-chunks. seg-outer
            # so only xT's first 512 columns gate the start of compute.
            # The fp8 hi/lo split of u (DoubleRow logitsT operand) is
            # emitted per segment so produce(0) finds its g=0 blocks ready.
            uT_sb = pers.tile([P, NC_D, S], BF16, tag="uT")
            uh_sb = pers.tile([P, NC_D, S], FP8, tag="uh")
            ul_sb = pers.tile([P, NC_D, S], FP8, tag="ul")

            def u_prep(g, ms=range(NC_D)):
                # fp8 hi/lo split of u seg g; deferred (in halves, to keep
                # ACT's per-window load flat) until shortly before
                # produce(2g) consumes it, so phase-1 engines stay clear
                sl = slice(g * SEG, (g + 1) * SEG)
                for m in ms:
                    nc.scalar.activation(
                        out=uh_sb[:, m, sl], in_=uT_sb[:, m, sl], func=Act.Identity
                    )
                    eng = nc.vector if m % 2 == 0 else nc.gpsimd
                    eng.tensor_sub(ul_sb[:, m, sl], uT_sb[:, m, sl], uh_sb[:, m, sl])

            for g in range(NSEG):
                sl = slice(g * SEG, (g + 1) * SEG)
                pss = [
                    psA.tile([P, SEG], F32, tag="mm", name=f"pj{m}")
                    for m in range(NC_D)
                ]
                n = 0
                for s_sb, x_sb in ((mh_sb, xh_sb), (mh_sb, xl_sb), (ml_sb, xh_sb)):
                    for cp in range(NC_D // 2):
                        cs = slice(2 * cp, 2 * cp + 2)
                        n += 1
                        for m in range(NC_D):
                            nc.tensor.matmul(
                                pss[m],
                                s_sb[:, cs, m * P : (m + 1) * P],
                                x_sb[:, cs, sl],
                                start=(n == 1),
                                stop=(n == 3 * (NC_D // 2)),
                                perf_mode=DR,
                            )
                for m in range(NC_D):
                    # evict + cast to bf16; alternate ACT/DVE so two
                    # engines drain PSUM
                    if m % 2 == 0:
                        nc.scalar.activation(
                            out=uT_sb[:, m, sl], in_=pss[m], func=Act.Identity
                        )
                    else:
                        nc.vector.tensor_copy(out=uT_sb[:, m, sl], in_=pss[m])
            u_prep(0)
            # gamma[k] = x[k,:] @ gk: 1-row matmuls, ~free on the hwdecode
            # PE; added per-partition as the exp eviction's bias
            gam_sb = pers.tile([P, NBLK], F32, tag="gam")
            gps = psA.tile([P, SEG], F32, tag="mm", name="gps")
            for kb in range(NBLK):
                n = 0
                for x_sb in (xh_sb, xl_sb):
                    for c in range(NC_D):
                        n += 1
                        nc.tensor.matmul(
                            gps[:, kb : kb + 1],
                            x_sb[:, c, kb * P : (kb + 1) * P],
                            gk_sb[:, c : c + 1],
                            start=(n == 1),
                            stop=(n == 2 * NC_D),
                        )
            nc.vector.tensor_copy(out=gam_sb, in_=gps[:, 0:NBLK])
            # v[s,d']: stationary = xT block [d, s-block], moving = Wv [d, d']
            # v is consumed by fp8 DoubleRow attn@v as hi+lo (error-
            # compensated fp8 split): v_hi = fp8(v), v_lo = fp8(v - v_hi).
            # Engines: DVE computes v (bias add), ACT casts hi, GPSIMD the
            # lo residual — all off the PE's critical path.
            v_sb = pers.tile([P, NBLK, D], BF16, tag="v")
            vh_sb = pers.tile([P, NBLK, D], FP8, tag="vh")
            vl_sb = pers.tile([P, NBLK, D], FP8, tag="vl")
            for j in range(NBLK):
                ps = psA.tile([P, SEG], F32, tag="mm", name="vps")
                n = 0
                for x_sb, w_sb in ((xh_sb, wvh_sb), (xh_sb, wvl_sb), (xl_sb, wvh_sb)):
                    for cp in range(NC_D // 2):
                        cs = slice(2 * cp, 2 * cp + 2)
                        n += 1
                        nc.tensor.matmul(
                            ps,
                            x_sb[:, cs, j * P : (j + 1) * P],
                            w_sb[:, cs, :],
                            start=(n == 1),
                            stop=(n == 3 * (NC_D // 2)),
                            perf_mode=DR,
                        )
                # evict: undo the wv prescale + bias along free dim + cast
                nc.vector.scalar_tensor_tensor(
                    out=v_sb[:, j, :],
                    in0=ps,
                    scalar=1.0 / WVSCALE,
                    in1=bv_bc,
                    op0=Alu.mult,
                    op1=Alu.add,
                )
                nc.scalar.activation(
                    out=vh_sb[:, j, :], in_=v_sb[:, j, :], func=Act.Identity
                )
                # GPSIMD alone would overrun the v-loop window; DVE takes
                # every 4th residual
                eng = nc.vector if j % 4 == 3 else nc.gpsimd
                eng.tensor_sub(vl_sb[:, j, :], v_sb[:, j, :], vh_sb[:, j, :])



            # ---- phase 2: attention + layernorm, per 256-column q pair ----
            # Software-pipelined: produce pair p+1 (logitsT+exp) before
            # consuming pair p (attn@v + LN epilogue), so the PE never waits
            # on the ACT exp latency.
            def produce(p):
                # logitsT[k, q] per 128-k-block: stationary = xT block,
                # moving = uT pair-chunk. exp(logitsT) lands in a bf16 tmp,
                # then splits into fp8 hi+lo for DoubleRow attn@v. Engine
                # split keeps each under the PE's 427ns/k-block pace:
                # ACT exp, DVE hi cast, GPSIMD lo residual.
                tmp = attnp.tile([P, NBLK, QP], BF16, tag="attnT")
                aH = attnHp.tile([P, NBLK, QP], FP8, tag="aH")
                aL = attnLp.tile([P, NBLK, QP], FP8, tag="aL")
                qsl = slice(p * QP, (p + 1) * QP)
                for kb in range(NBLK):
                    lg = psA.tile([P, SEG], F32, tag="mm", name=f"lg{kb % 5}")
                    ksl = slice(kb * P, (kb + 1) * P)
                    n = 0
                    for cp in range(NC_D // 2):
                        cs = slice(2 * cp, 2 * cp + 2)
                        for sx, su in (
                            (xh_sb, uh_sb),
                            (xh_sb, ul_sb),
                            (xl_sb, uh_sb),
                        ):
                            n += 1
                            nc.tensor.matmul(
                                lg[:, 0:QP],
                                sx[:, cs, ksl],
                                su[:, cs, qsl],
                                start=(n == 1),
                                stop=(n == 3 * (NC_D // 2)),
                                perf_mode=DR,
                            )
                    # no max subtraction (|logits| < ~2.5 for this problem);
                    # gamma carries the k-bias term (zero for zero bq).
                    # M (hence u and the psum logits) is host-scaled by 64
                    # to keep u's fp8 hi/lo split out of the subnormal
                    # range; the exp's scale undoes it.
                    nc.scalar.activation(
                        out=tmp[:, kb, :],
                        in_=lg[:, 0:QP],
                        func=Act.Exp,
                        bias=gam_sb[:, kb : kb + 1],
                        scale=1.0 / USCALE,
                    )
                # hi/lo splits after the matmul+exp loop: ACT's exps keep
                # the psA-bank recycling pace, the casts drain afterwards
                # (deadline is the consume, a full pair-period later)
                for kb in range(NBLK):
                    if kb % 2 == 0 and p >= 2:
                        nc.gpsimd.tensor_copy(out=aH[:, kb, :], in_=tmp[:, kb, :])
                    elif kb % 2 == 0:
                        # GPSIMD still drains the phase-1 v residuals during
                        # the first two pairs — DVE takes its share there
                        nc.vector.tensor_copy(out=aH[:, kb, :], in_=tmp[:, kb, :])
                    else:
                        nc.scalar.activation(
                            out=aH[:, kb, :], in_=tmp[:, kb, :], func=Act.Identity
                        )
                    eng = nc.gpsimd if (p < 2 and kb % 2 == 0) else nc.vector
                    eng.tensor_sub(aL[:, kb, :], tmp[:, kb, :], aH[:, kb, :])
                return tmp, aH, aL

            # ---- epilogue, split in two stages ----
            # softmax normalization folded into LN:
            #   raw = attn_unnorm @ v; normalized x = raw / rowsum
            #   out = (raw - mean_raw) * c1 * gamma + beta, where
            #   c1 = (var_raw + eps*rowsum^2)^-0.5
            # (equals rstd(x)/rowsum analytically; eps*rowsum^2 keeps the
            # torch eps semantics). Stage A (DVE stats) is emitted with the
            # consume; stage B (ACT rsqrt via Exp(-0.5*Ln), final pass,
            # store) is deferred until after the NEXT produce so the ACT
            # FIFO never blocks that pair's exp evictions behind a
            # DVE-dependent Ln.
            def epi_a(p, j, out_ps, sums):
                sc = small.tile([P, 1], F32, tag="sc")
                nc.vector.tensor_copy(out=sc, in_=sums[:, j : j + 1])
                bst = small.tile([P, 6], F32, tag="bst")
                nc.vector.bn_stats(out=bst, in_=out_ps)
                mv = small.tile([P, 2], F32, tag="mv")
                nc.vector.bn_aggr(out=mv, in_=bst)
                t = small.tile([P, 1], F32, tag="t")
                nc.vector.tensor_scalar(
                    out=t,
                    in0=sc,
                    scalar1=sc,
                    scalar2=float(EPS),
                    op0=Alu.mult,
                    op1=Alu.mult,
                )
                return mv, t

            def epi_b(p, j, out_ps, mv, t, split, alt_queue=False):
                # rstd = (var + eps*s^2)^-0.5 as Exp(-0.5*Ln(.)) — the ACT
                # engine stays on the single ln+exp function table (a Sqrt
                # would force a 1.3us table reload twice per chunk)
                lnv = small.tile([P, 1], F32, tag="lnv")
                nc.scalar.activation(
                    out=lnv, in_=mv[:, 1:2], func=Act.Ln, bias=t, scale=1.0
                )
                c1 = small.tile([P, 1], F32, tag="c1")
                nc.scalar.activation(out=c1, in_=lnv, func=Act.Exp, scale=-0.5)

                row = (p * 2 + j) * P
                hw_ = D // split
                for h in range(split):
                    cols = slice(h * hw_, (h + 1) * hw_)
                    y = work.tile([P, hw_], F32, tag=f"y{h}", name=f"y{h}")
                    nc.vector.tensor_scalar(
                        out=y,
                        in0=out_ps[:, cols],
                        scalar1=mv[:, 0:1],
                        scalar2=c1,
                        op0=Alu.subtract,
                        op1=Alu.mult,
                    )
                    if not g1b0:
                        o1 = work.tile([P, hw_], F32, tag=f"o1{h}", name=f"o1{h}")
                        nc.vector.tensor_mul(o1, y, gamma_bc[:, cols])
                        y = work.tile([P, hw_], F32, tag=f"o{h}", name=f"o{h}")
                        nc.vector.tensor_add(y, o1, beta_bc[:, cols])
                    # alternate trigger queues on the tail so the final
                    # stores issue in parallel instead of serializing on SP
                    eng = nc.scalar if (alt_queue and h % 2 == 1) else nc.sync
                    eng.dma_start(out=out_d.ap()[row : row + P, cols], in_=y)

            # one persistent sums bank, column-region double-buffered by pair
            # parity so consecutive pairs' rowsum accumulations never share a
            # WAR dependency on the epilogue's read
            sums_all = psS.tile([P, 6], F32, tag="s")

            # attn@v in fp8 DoubleRow: each matmul contracts 256 k (two
            # adjacent kb blocks packed 2-per-partition, which is exactly
            # the layout of the [P, kb, ...] tiles) at 0.5 cycles/row.
            # Error-compensated: hi*vh + hi*vl + lo*vh ~ bf16 accuracy at
            # 0.75x the bf16 PE cost. Rowsums via 1-row DoubleRow matmuls
            # of (hi + lo) — exactly the weights the products apply.
            NK2 = NBLK // 2

            def consume_mm(p, tmp, aH, aL, outp, sums_col, j, cols=slice(0, D)):
                for kb2 in range(NK2):
                    ksl = slice(2 * kb2, 2 * kb2 + 2)
                    stH = aH[:, ksl, j * P : (j + 1) * P]
                    stL = aL[:, ksl, j * P : (j + 1) * P]
                    first = kb2 == 0
                    last = kb2 == NK2 - 1
                    nc.tensor.matmul(
                        outp, stH, vh_sb[:, ksl, cols],
                        start=first, stop=False, perf_mode=DR,
                    )
                    nc.tensor.matmul(
                        outp, stH, vl_sb[:, ksl, cols],
                        start=False, stop=False, perf_mode=DR,
                    )
                    nc.tensor.matmul(
                        outp, stL, vh_sb[:, ksl, cols],
                        start=False, stop=last, perf_mode=DR,
                    )
                    if sums_col is not None:
                        for i in (0, 1):
                            nc.tensor.matmul(
                                sums_col,
                                tmp[:, 2 * kb2 + i, j * P : (j + 1) * P],
                                ones_sb,
                                start=first and i == 0,
                                stop=last and i == 1,
                            )

            def consume_a(p, tmp, aH, aL):
                outps = [
                    psO.tile([P, D], F32, tag="out", name=f"out{j}") for j in (0, 1)
                ]
                sums = sums_all[:, (p % 2) * 2 : (p % 2) * 2 + 2]
                for j in (0, 1):
                    consume_mm(p, tmp, aH, aL, outps[j], sums[:, j : j + 1], j)
                state = []
                for j in (0, 1):
                    mv, t = epi_a(p, j, outps[j], sums)
                    state.append((outps[j], mv, t))
                return state

            # epi_b(p-2) is emitted BEFORE produce(p) so its psO-releasing
            # DVE reads run at the head of the produce window instead of
            # queueing behind the pair's hi/lo splits — consume(p-1) would
            # otherwise stall on the psO buffer WAR
            pend_attn = None  # produce(p) output awaiting consume
            pend_epi = None  # (p, state) awaiting epi_b
            for p in range(NPAIR):
                if pend_epi is not None:
                    ep, st = pend_epi
                    for j in (0, 1):
                        epi_b(ep, j, st[j][0], st[j][1], st[j][2], split=1)
                    pend_epi = None
                produced = produce(p)
                # u seg fp8 splits, half a segment per produce window so
                # ACT's cast load stays flat; emitted after produce so the
                # casts queue behind this pair's exps
                if p + 2 < NPAIR and p % 2 == 0:
                    u_prep((p + 2) // 2, range(0, 2))
                if p + 1 < NPAIR and (p + 1) % 2 == 0:
                    u_prep((p + 1) // 2, range(2, NC_D))
                if pend_attn is not None:
                    pend_epi = (p - 1, consume_a(p - 1, *pend_attn))
                pend_attn = produced
            ep, st = pend_epi
            for j in (0, 1):
                epi_b(ep, j, st[j][0], st[j][1], st[j][2], split=1)

            # last pair: accumulate into now-idle psA banks (no WAR against
            # the previous pair's psO epilogue reads), run the two q-chunks
            # back-to-back so chunk j=1's full epilogue+store overlaps chunk
            # j=0's matmuls, and column-halve j=0's accumulation so its
            # bn_stats mostly overlaps the final matmuls
            pl = NPAIR - 1
            tmpT, aH, aL = pend_attn
            sums = sums_all[:, (pl % 2) * 2 : (pl % 2) * 2 + 2]
            lout1 = psA.tile([P, D], F32, tag="mm", name="lout1")
            consume_mm(pl, tmpT, aH, aL, lout1, sums[:, 1:2], 1)
            mv, t = epi_a(pl, 1, lout1, sums)
            epi_b(pl, 1, lout1, mv, t, split=2)

            # j=0 accumulates its two column halves into SEPARATE tiles:
            # tile-level dependency tracking would otherwise see the h0
            # bn_stats (emitted between the halves so it overlaps the h1
            # matmuls) as conflicting with the h1 writes and stall the PE
            lsums = sums_all[:, 4:5]  # untouched column: no tracked deps
            # asymmetric halves: the small trailing piece minimizes the
            # post-PE bn_stats and the final store's transfer time
            HSPLIT = (slice(0, 384), slice(384, D))
            louts = [
                psA.tile([P, 384 if h == 0 else D - 384], F32, tag="mm", name=f"l0h{h}")
                for h in (0, 1)
            ]
            bst2 = small.tile([P, 12], F32, tag="bst2")
            for h in (0, 1):
                cols = HSPLIT[h]
                consume_mm(pl, tmpT, aH, aL, louts[h], lsums if h == 0 else None, 0, cols)
                if h == 0:
                    sc = small.tile([P, 1], F32, tag="sc")
                    nc.vector.tensor_copy(out=sc, in_=lsums)
                    t = small.tile([P, 1], F32, tag="t")
                    nc.vector.tensor_scalar(
                        out=t,
                        in0=sc,
                        scalar1=sc,
                        scalar2=float(EPS),
                        op0=Alu.mult,
                        op1=Alu.mult,
                    )
                nc.vector.bn_stats(out=bst2[:, h * 6 : (h + 1) * 6], in_=louts[h])
            mv = small.tile([P, 2], F32, tag="mv")
            nc.vector.bn_aggr(out=mv, in_=bst2)
            lnv = small.tile([P, 1], F32, tag="lnv")
            nc.scalar.activation(
                out=lnv, in_=mv[:, 1:2], func=Act.Ln, bias=t, scale=1.0
            )
            c1 = small.tile([P, 1], F32, tag="c1")
            nc.scalar.activation(out=c1, in_=lnv, func=Act.Exp, scale=-0.5)
            row = pl * 2 * P
            for h in (0, 1):
                cols = HSPLIT[h]
                hw_ = cols.stop - cols.start
                y = work.tile([P, hw_], F32, tag=f"y{h}", name=f"ly{h}")
                nc.vector.tensor_scalar(
                    out=y,
                    in0=louts[h],
                    scalar1=mv[:, 0:1],
                    scalar2=c1,
                    op0=Alu.subtract,
                    op1=Alu.mult,
                )
                if not g1b0:
                    o1 = work.tile([P, hw_], F32, tag=f"o1{h}", name=f"lo1{h}")
                    nc.vector.tensor_mul(o1, y, gamma_bc[:, cols])
                    y = work.tile([P, hw_], F32, tag=f"o{h}", name=f"lo{h}")
                    nc.vector.tensor_add(y, o1, beta_bc[:, cols])
                nc.sync.dma_start(out=out_d.ap()[row : row + P, cols], in_=y)

    # Force every ACT instruction onto the one table set that contains all
    # functions we use ({exp, ln, identity} ⊆ natural_log_exp_and_others).
    # The default chooser picks the FIRST set containing each function
    # (exp→set0, ln→set5), inserting a 1.28us table reload twice per
    # chunk. Entries must keep their positions (act_func_set_id is the
    # index), so unwanted sets are emptied rather than removed.
    import concourse.bacc as bacc_mod

    orig_get_tables = bacc_mod.get_activation_tables

    def pinned_tables(arch):
        out = {}
        for name, funcs in orig_get_tables(arch).items():
            out[name] = funcs if name == "natural_log_exp_and_others" else set()
        return out

    bacc_mod.get_activation_tables = pinned_tables
    try:
        nc.compile()
    finally:
        bacc_mod.get_activation_tables = orig_get_tables
    return nc


def _numpy_fallback(query, mask, Wq, bq, Wk, bk, Wv, bv, gamma, beta):
    q = query @ Wq + bq
    k = query @ Wk + bk
    v = query @ Wv + bv
    scale = 1.0 / np.sqrt(np.float32(q.shape[-1]))
    logits = np.einsum("bqd,bkd->bqk", q, k) * scale
    m = np.swapaxes(mask, 1, 2)
    logits = np.where(m, logits, np.float32(-1e9))
    logits = logits - logits.max(axis=2, keepdims=True)
    attn = np.exp(logits)
    attn = attn / attn.sum(axis=2, keepdims=True)
    out = np.einsum("bqk,bkd->bqd", attn, v)
    mu = out.mean(axis=-1, keepdims=True)
    var = out.var(axis=-1, keepdims=True)
    return (out - mu) / np.sqrt(var + 1e-5) * gamma + beta


def kernel(query, mask, Wq, bq, Wk, bk, Wv, bv, gamma, beta):
    global last_results
    from concourse.bass_utils import run_bass_kernel_spmd

    query = np.asarray(query, dtype=np.float32)
    mask = np.asarray(mask)
    Wq = np.asarray(Wq, dtype=np.float32)
    Wk = np.asarray(Wk, dtype=np.float32)
    Wv = np.asarray(Wv, dtype=np.float32)
    bq = np.asarray(bq, dtype=np.float32)
    bk = np.asarray(bk, dtype=np.float32)
    bv = np.asarray(bv, dtype=np.float32)
    gamma = np.asarray(gamma, dtype=np.float32)
    beta = np.asarray(beta, dtype=np.float32)

    if not mask.all():
        # General-mask path (never hit for this problem's all-ones mask).
        return _numpy_fallback(
            query, mask, Wq, bq, Wk, bk, Wv, bv, gamma, beta
        ).astype(np.float32)

    g1b0 = bool((gamma == 1.0).all() and (beta == 0.0).all())
    if g1b0 not in _cached_nc:
        _cached_nc[g1b0] = _build_nc(g1b0)
    nc = _cached_nc[g1b0]

    F8 = ml_dtypes.float8_e4m3fn

    def split8(a):
        hi = a.astype(F8)
        lo = (a - hi.astype(np.float64)).astype(F8)
        return hi, lo

    scale = 1.0 / np.sqrt(np.float64(D))
    # fold the two q/k projections into one: logits = x @ m @ x^T + gk-term
    m_f = (Wq.astype(np.float64) @ Wk.astype(np.float64).T) * (scale * USCALE)
    mh_b, ml_b = split8(m_f)
    wvh_b, wvl_b = split8(Wv.astype(np.float64) * WVSCALE)
    gk_b = ((Wk.astype(np.float64) @ bq.astype(np.float64)) * scale).astype(BF)

    in_maps = []
    for b in range(B):
        xT = np.ascontiguousarray(query[b].T).astype(np.float64)
        xh, xl = split8(xT)
        m = {
            "xh": xh,
            "xl": xl,
            "mh": mh_b,
            "ml": ml_b,
            "wvh": wvh_b,
            "wvl": wvl_b,
            "gk": gk_b,
            "bv": bv,
        }
        if not g1b0:
            m["gamma"] = gamma
            m["beta"] = beta
        in_maps.append(m)

    res = run_bass_kernel_spmd(nc, in_maps, core_ids=list(range(B)))
    last_results = res
    out = np.stack([res.results[b]["out"] for b in range(B)], axis=0)
    return out.astype(np.float32)
